# revision 20
# baseline (speedup 1.0000x reference)
"""Trainium2 Bass kernel for causal multi-head attention with adaptive
temperature (entropy-polynomial) softmax.

Problem shape: x [2, 2048, 1024], 16 heads x 64 dims, causal.
  q/k/v = x @ W{q,k,v}.T ; sim = q k^T / 8 (causal) ;
  attn = softmax(beta * sim), beta = f(entropy(softmax(sim))) ;
  out = (attn v) @ Wo.T + bo

Sharding (8 cores): core c owns batch b = c // 4 and heads
4*(c%4) .. 4*(c%4)+3.  Host sums the partials per batch and adds bo.

Device-side per core (m indexes head pairs {2m, 2m+1}):
  phase A   : qT16/kT16 = (W slice) @ x^T -> fp16 (qT pre-scaled 1/8);
              v packed fp16 + ones column per head (v_aug); x/W arrive
              fp16 from the host (halves input DMA)
  B1(m)     : entropy-stat sweep in [i, j] over [128,1024] psum tiles:
              scores -> exp (accum Z1) -> l*exp(l) (accum D)
  stats(m)  : H = ln Z1 - D/Z1 ; beta = where(H>.5, max(poly(H),1), 1)
  fold(m)   : PE transpose of beta + SBUF DMA gather + partition
              broadcast; qbT = qT16 * beta (fp16).  The transpose is
              emitted mid-way through the next PE-busy sweep so the
              cross-engine fold chain never stalls the PE.
  B2(m)     : transposed pass, group-outer: per group g (512 query
              cols) and head hh: scoresT[j,i] = kT16^T @ qbT (pairs of
              j blocks share one [128,1024] psum tile -> one exp),
              maskT on diagonal, exp -> e2 fp16, avT[65,512] +=
              v_aug^T @ e2 (row 64 = Z2 via ones col).  Per-group
              normalize (DMA-reshaped partition-parallel reciprocal)
              then the group's 4 rows of the output projection
              partial{m} = attT[m].T @ Wo^T stream out immediately.
Host sums partial0+partial1 over the 4 cores of each batch.
"""

import numpy as np

import concourse.bass as bass
import concourse.tile as tile
from concourse import bacc, mybir
from concourse.bass_utils import run_bass_kernel_spmd
from concourse.masks import make_identity

F32 = mybir.dt.float32
F32R = mybir.dt.float32r
BF16 = mybir.dt.bfloat16
FP16 = mybir.dt.float16
I32 = mybir.dt.int32
AFT = mybir.ActivationFunctionType
ALU = mybir.AluOpType

B, N, DIM = 2, 2048, 1024
H_TOT, HD = 16, 64
N_CORES = 8
NH = 4            # heads per core
CD = NH * HD      # 256 channel dims per core
NRB = N // 128    # 16 row blocks
NG = NRB // 4     # 4 groups of 4 row blocks (512 query cols each)
POLY = [-0.037, 0.481, -2.3, 4.917, -1.791]
MASK_VAL = -1e30
SCALE = 1.0 / 8.0  # 1/sqrt(64)

NUC = NRB * 2        # 32 (rb, head-in-pair) units per m


def build_kernel():
    nc = bacc.Bacc("TRN2", target_bir_lowering=False, debug=False,
                   num_devices=N_CORES)

    xT = nc.dram_tensor("xT", [DIM, N], FP16, kind="ExternalInput").ap()
    wqT = nc.dram_tensor("wqT", [DIM, CD], FP16, kind="ExternalInput").ap()
    wkT = nc.dram_tensor("wkT", [DIM, CD], FP16, kind="ExternalInput").ap()
    wvT = nc.dram_tensor("wvT", [DIM, CD], FP16, kind="ExternalInput").ap()
    woT = nc.dram_tensor("woT", [CD, DIM], F32, kind="ExternalInput").ap()
    maskin = nc.dram_tensor("maskin", [128, 128], F32, kind="ExternalInput").ap()
    maskTin = nc.dram_tensor("maskTin", [128, 128], F32, kind="ExternalInput").ap()
    partial = [nc.dram_tensor(f"partial{m}", [N, DIM], F32,
                              kind="ExternalOutput").ap() for m in range(2)]

    KC = DIM // 128  # 8 contraction chunks

    with tile.TileContext(nc) as tc:
        with tc.tile_pool(name="const", bufs=1) as constp, \
             tc.tile_pool(name="qkv_sb", bufs=1) as qkvp, \
             tc.tile_pool(name="attn_out", bufs=1) as aop, \
             tc.tile_pool(name="wo_sb", bufs=1) as wop, \
             tc.tile_pool(name="statsall", bufs=1) as sap:

            identf = constp.tile([128, 128], F32)
            make_identity(nc, identf[:])
            mask = constp.tile([128, 128], F32)
            nc.sync.dma_start(mask[:], maskin[:])
            maskT = constp.tile([128, 128], F32)
            nc.sync.dma_start(maskT[:], maskTin[:])
            ones32 = constp.tile([128, NUC], F32)
            nc.vector.memset(ones32[:], 1.0)

            qT16 = [qkvp.tile([128, N], FP16, tag=f"qT{m}", name=f"qT{m}") for m in range(2)]
            kT16 = [qkvp.tile([128, N], FP16, tag=f"kT{m}", name=f"kT{m}") for m in range(2)]
            vaug = [qkvp.tile([128, NH * 65], FP16, tag=f"va{j}", name=f"va{j}") for j in range(NRB)]
            qbT = [qkvp.tile([128, N], FP16, tag=f"qb{m}", name=f"qb{m}") for m in range(2)]
            attT = [aop.tile([128, N], F32R, tag=f"attT{m}", name=f"attT{m}") for m in range(2)]
            woS = [wop.tile([128, DIM], F32R, tag=f"wo{m}", name=f"wo{m}") for m in range(2)]

            # per-m stats accumulators: col = (rb*2 + hh)*2 + chunk
            Z1p = [sap.tile([128, 2 * NUC], F32, tag=f"Z1p{m}", name=f"Z1p{m}") for m in range(2)]
            D1p = [sap.tile([128, 2 * NUC], F32, tag=f"D1p{m}", name=f"D1p{m}") for m in range(2)]

            # ---- phase A: QKV projections ----
            with tc.tile_pool(name="xw_sb", bufs=1) as xwp, \
                 tc.tile_pool(name="qkv_ps", bufs=4, space="PSUM") as qkps:
                xTs = [xwp.tile([128, N], FP16, tag=f"xT{k}", name=f"xTs{k}") for k in range(KC)]
                wq_s = [xwp.tile([128, CD], FP16, tag=f"wq{k}", name=f"wq{k}") for k in range(KC)]
                wk_s = [xwp.tile([128, CD], FP16, tag=f"wk{k}", name=f"wk{k}") for k in range(KC)]
                wv_s = [xwp.tile([128, CD], FP16, tag=f"wv{k}", name=f"wv{k}") for k in range(KC)]
                for k in range(KC):
                    sl = slice(128 * k, 128 * (k + 1))
                    nc.sync.dma_start(wq_s[k][:], wqT[sl, :])
                    nc.sync.dma_start(wk_s[k][:], wkT[sl, :])
                    nc.sync.dma_start(wv_s[k][:], wvT[sl, :])
                    nc.sync.dma_start(xTs[k][:], xT[sl, :])
                for m in range(2):
                    nc.sync.dma_start(woS[m][:], woT[128 * m:128 * (m + 1), :].bitcast(F32R))

                for m in range(2):
                    for which, wt, dest, scl in (("q", wq_s, qT16, SCALE), ("k", wk_s, kT16, 1.0)):
                        for nn in range(N // 512):
                            pq = qkps.tile([128, 512], F32, tag="pq")
                            for k in range(KC):
                                nc.tensor.matmul(
                                    pq[:], wt[k][:, 128 * m:128 * (m + 1)],
                                    xTs[k][:, 512 * nn:512 * (nn + 1)],
                                    start=(k == 0), stop=(k == KC - 1))
                            if which == "q":
                                nc.scalar.activation(
                                    dest[m][:, 512 * nn:512 * (nn + 1)], pq[:],
                                    AFT.Copy, bias=0.0, scale=scl)
                            else:
                                nc.vector.tensor_copy(
                                    dest[m][:, 512 * nn:512 * (nn + 1)], pq[:])

                for jt in range(NRB):
                    pv = qkps.tile([128, CD], F32, tag="pv")
                    for k in range(KC):
                        nc.tensor.matmul(
                            pv[:], xTs[k][:, 128 * jt:128 * (jt + 1)], wv_s[k][:],
                            start=(k == 0), stop=(k == KC - 1))
                    va3 = vaug[jt][:].rearrange("p (h x) -> p h x", x=65)
                    nc.any.tensor_copy(
                        va3[:, :, 0:64],
                        pv[:].rearrange("p (h x) -> p h x", x=64))
                    nc.gpsimd.memset(va3[:, :, 64:65], 1.0)

            # ---- pipelined B1 / stats / fold / B2+C ----
            with tc.tile_pool(name="scr", bufs=4) as scrp, \
                 tc.tile_pool(name="stats", bufs=2) as stp, \
                 tc.tile_pool(name="bprep", bufs=2) as bpp, \
                 tc.tile_pool(name="e2sb", bufs=6) as e2p, \
                 tc.tile_pool(name="ostp", bufs=3) as ostp, \
                 tc.tile_pool(name="sc_ps", bufs=3, space="PSUM") as scps, \
                 tc.tile_pool(name="av_ps", bufs=2, space="PSUM") as avps:

                beta_sb = [None, None]

                def b1_gen(m):
                    nc.vector.memset(Z1p[m][:], 0.0)
                    nc.vector.memset(D1p[m][:], 0.0)
                    for rb in range(NRB):
                        yield
                        W = 128 * (rb + 1)
                        tiles = [(o, min(1024, W - o)) for o in range(0, W, 1024)]
                        for hh in range(2):
                            base = 64 * hh
                            col = (rb * 2 + hh) * 2
                            q_l = qT16[m][base:base + 64, 128 * rb:128 * (rb + 1)]
                            for ci, (off, tw) in enumerate(tiles):
                                ps = scps.tile([128, 1024], F32, tag="ps_s")
                                for o2 in range(0, tw, 512):
                                    sw = min(512, tw - o2)
                                    nc.tensor.matmul(
                                        ps[:, o2:o2 + sw], q_l,
                                        kT16[m][base:base + 64,
                                                off + o2:off + o2 + sw],
                                        start=True, stop=True)
                                if off + tw == W:
                                    nc.vector.tensor_tensor(
                                        out=ps[:, tw - 128:tw],
                                        in0=ps[:, tw - 128:tw],
                                        in1=mask[:], op=ALU.add)
                                t1 = scrp.tile([128, 1024], F32, tag="t1")
                                nc.scalar.activation(
                                    t1[:, :tw], ps[:, :tw], AFT.Exp,
                                    bias=0.0, scale=1.0,
                                    accum_out=Z1p[m][:, col + ci:col + ci + 1])
                                s2 = scrp.tile([128, 1024], F32, tag="s2")
                                nc.vector.scalar_tensor_tensor(
                                    out=s2[:, :tw], in0=ps[:, :tw], scalar=1.0,
                                    in1=t1[:, :tw], op0=ALU.mult, op1=ALU.mult,
                                    accum_out=D1p[m][:, col + ci:col + ci + 1])

                def stats_chain(m):
                    Z1a = stp.tile([128, NUC], F32, tag="Z1a")
                    D1a = stp.tile([128, NUC], F32, tag="D1a")
                    nc.vector.tensor_reduce(
                        out=Z1a[:], in_=Z1p[m].rearrange("p (u c) -> p u c", c=2),
                        axis=mybir.AxisListType.X, op=ALU.add)
                    nc.vector.tensor_reduce(
                        out=D1a[:], in_=D1p[m].rearrange("p (u c) -> p u c", c=2),
                        axis=mybir.AxisListType.X, op=ALU.add)
                    rz = stp.tile([128, NUC], F32, tag="rz")
                    nc.vector.reciprocal(rz[:], Z1a[:])
                    dn = stp.tile([128, NUC], F32, tag="dn")
                    nc.vector.tensor_mul(dn[:], D1a[:], rz[:])
                    lnz = stp.tile([128, NUC], F32, tag="lnz")
                    nc.scalar.activation(lnz[:], Z1a[:], AFT.Ln, bias=0.0, scale=1.0)
                    Hent = stp.tile([128, NUC], F32, tag="Hent")
                    nc.vector.tensor_sub(Hent[:], lnz[:], dn[:])
                    p0 = stp.tile([128, NUC], F32, tag="p0")
                    nc.vector.tensor_scalar(out=p0[:], in0=Hent[:], scalar1=POLY[0],
                                            scalar2=POLY[1], op0=ALU.mult, op1=ALU.add)
                    p1 = stp.tile([128, NUC], F32, tag="p1")
                    for c in POLY[2:]:
                        nc.vector.tensor_mul(p1[:], p0[:], Hent[:])
                        nc.vector.tensor_scalar_add(p0[:], p1[:], c)
                    nc.vector.tensor_scalar_max(p1[:], p0[:], 1.0)
                    mk = stp.tile([128, NUC], I32, tag="mk")
                    nc.vector.tensor_scalar(out=mk[:], in0=Hent[:], scalar1=0.5,
                                            scalar2=None, op0=ALU.is_gt)
                    beta_m = stp.tile([128, NUC], F32, tag="beta_m")
                    nc.vector.tensor_copy(beta_m[:], ones32[:])
                    nc.vector.copy_predicated(beta_m[:], mk[:], p1[:])
                    beta_sb[m] = beta_m

                def fold(m):
                    beta_m = beta_sb[m]
                    btp = scps.tile([128, 1024], F32, tag="ps_s")
                    nc.tensor.transpose(btp[0:NUC, 0:128], beta_m[:], identf[:])
                    betaT = bpp.tile([NUC, 128], F32, tag="betaT")
                    nc.any.tensor_copy(betaT[:], btp[0:NUC, 0:128])
                    for hh in range(2):
                        base = 64 * hh
                        brow = bpp.tile([1, N], F32, tag="brow")
                        nc.sync.dma_start(brow[:], betaT[hh::2, :])
                        bb = bpp.tile([128, N], F32, tag="bb")
                        nc.gpsimd.partition_broadcast(bb[:], brow[:])
                        nc.vector.tensor_tensor(
                            out=qbT[m][base:base + 64, :],
                            in0=qT16[m][base:base + 64, :],
                            in1=bb[base:base + 64, :], op=ALU.mult)

                def proj_rows(m, g):
                    for rb in range(4 * g, 4 * g + 4):
                        pp = scps.tile([128, 1024], F32, tag="ps_s")
                        for nn in range(2):
                            nc.tensor.matmul(
                                pp[:, 512 * nn:512 * (nn + 1)],
                                attT[m][:, 128 * rb:128 * (rb + 1)],
                                woS[m][:, 512 * nn:512 * (nn + 1)],
                                start=True, stop=True)
                        ost = ostp.tile([128, 1024], F32, tag="ost")
                        nc.vector.tensor_copy(ost[:], pp[:])
                        nc.sync.dma_start(
                            partial[m][128 * rb:128 * (rb + 1), :], ost[:])

                def b2_gen(m):
                    for g in range(NG):
                        i0 = 512 * g
                        njt = 4 * g + 4
                        zall = stp.tile([1, 1024], F32, tag="zall")
                        for hh in range(2):
                            yield
                            h = 2 * m + hh
                            base = 64 * hh
                            avp = avps.tile([65, 512], F32, tag="avp")
                            for b0 in range(0, njt, 4):
                                bjts = list(range(b0, min(b0 + 4, njt)))
                                e2s = {}
                                # pairs of j-blocks share one psum tile and
                                # one exp instruction
                                for pi in range(0, len(bjts), 2):
                                    pjts = bjts[pi:pi + 2]
                                    psT = scps.tile([128, 1024], F32, tag="ps_s")
                                    ew = e2p.tile([128, 1024], FP16, tag="e2")
                                    hi = 0
                                    for sub, jt in enumerate(pjts):
                                        off = max(0, 128 * (jt - 4 * g))
                                        w = 512 - off
                                        s0 = 512 * sub
                                        nc.tensor.matmul(
                                            psT[:, s0:s0 + w],
                                            kT16[m][base:base + 64,
                                                    128 * jt:128 * (jt + 1)],
                                            qbT[m][base:base + 64,
                                                   i0 + off:i0 + 512],
                                            start=True, stop=True)
                                        if jt >= 4 * g:
                                            nc.vector.tensor_tensor(
                                                out=psT[:, s0:s0 + 128],
                                                in0=psT[:, s0:s0 + 128],
                                                in1=maskT[:], op=ALU.add)
                                        e2s[jt] = (ew, s0, off, w)
                                        hi = s0 + w
                                    nc.scalar.activation(
                                        ew[:, 0:hi], psT[:, 0:hi], AFT.Exp,
                                        bias=0.0, scale=1.0)
                                for jt in bjts:
                                    ew, s0, off, w = e2s[jt]
                                    nc.tensor.matmul(
                                        avp[:, off:512],
                                        vaug[jt][:, 65 * h:65 * h + 65],
                                        ew[:, s0:s0 + w],
                                        start=(jt == 0), stop=(jt == njt - 1),
                                        skip_group_check=True)
                            nc.scalar.copy(
                                zall[0:1, 512 * hh:512 * (hh + 1)], avp[64:65, :])
                            avps_held[hh] = avp
                        # per-group normalize: reshape Z rows to [128, 8] via
                        # DMA for a partition-parallel reciprocal; multiply
                        # straight from the held psum into attT
                        zres = stp.tile([128, 8], F32, tag="zres")
                        nc.sync.dma_start(zres[:], zall[:])
                        zrr = stp.tile([128, 8], F32, tag="zrr")
                        nc.vector.reciprocal(zrr[:], zres[:])
                        zr = stp.tile([1, 1024], F32, tag="zr")
                        nc.sync.dma_start(zr[:], zrr[:])
                        for hh in range(2):
                            base = 64 * hh
                            rbv = stp.tile([128, 512], F32, tag="rbv")
                            nc.gpsimd.partition_broadcast(
                                rbv[:], zr[0:1, 512 * hh:512 * (hh + 1)])
                            nc.vector.tensor_tensor(
                                out=attT[m][base:base + 64, i0:i0 + 512],
                                in0=avps_held[hh][0:64, :],
                                in1=rbv[base:base + 64, :], op=ALU.mult)

                avps_held = [None, None]

                def drain(gen, n=None):
                    i = 0
                    for _ in gen:
                        i += 1
                        if n is not None and i >= n:
                            return True
                    return False

                b1a = b1_gen(0)
                drain(b1a)
                stats_chain(0)
                b1b = b1_gen(1)
                drain(b1b, 4)          # fold(0) chain resolves under these
                fold(0)
                b2a = b2_gen(0)
                # alternate the remaining 12 B1 row blocks (ACT/DVE heavy)
                # with the 8 B2(0) units (PE heavy)
                alive_b1 = alive_b2 = True
                while alive_b1 or alive_b2:
                    if alive_b1:
                        alive_b1 = drain(b1b, 1)
                    if alive_b2:
                        alive_b2 = drain(b2a, 1)
                stats_chain(1)
                fold(1)
                b2b = b2_gen(1)
                for g in range(NG):
                    drain(b2b, 2)      # the two hh units of group g
                    proj_rows(0, g)
                drain(b2b)
                for g in range(NG):
                    proj_rows(1, g)

    nc.compile()
    return nc


_NC_CACHE = None
_LAST_IN_MAPS = None


def kernel(x, Wq, Wk, Wv, Wo, bo):
    global _NC_CACHE, _LAST_IN_MAPS
    x = np.asarray(x, dtype=np.float32)
    Wq = np.asarray(Wq, dtype=np.float32)
    Wk = np.asarray(Wk, dtype=np.float32)
    Wv = np.asarray(Wv, dtype=np.float32)
    Wo = np.asarray(Wo, dtype=np.float32)
    bo = np.asarray(bo, dtype=np.float32)

    if _NC_CACHE is None:
        _NC_CACHE = build_kernel()
    nc = _NC_CACHE

    mask_h = np.where(np.arange(128)[None, :] > np.arange(128)[:, None],
                      np.float32(MASK_VAL), np.float32(0.0)).astype(np.float32)
    maskT_h = np.ascontiguousarray(mask_h.T)
    woT_full = np.ascontiguousarray(Wo.T)  # [c, o]

    in_maps = []
    for c in range(N_CORES):
        b = c // 4
        s0 = CD * (c % 4)
        sl = slice(s0, s0 + CD)
        in_maps.append({
            "xT": np.ascontiguousarray(x[b].T.astype(np.float16)),
            "wqT": np.ascontiguousarray(Wq[sl, :].T.astype(np.float16)),
            "wkT": np.ascontiguousarray(Wk[sl, :].T.astype(np.float16)),
            "wvT": np.ascontiguousarray(Wv[sl, :].T.astype(np.float16)),
            "woT": np.ascontiguousarray(woT_full[sl, :]),
            "maskin": mask_h,
            "maskTin": maskT_h,
        })

    _LAST_IN_MAPS = in_maps
    res = run_bass_kernel_spmd(nc, in_maps, core_ids=list(range(N_CORES)))

    out = np.zeros((B, N, DIM), dtype=np.float32)
    for c in range(N_CORES):
        out[c // 4] += res.results[c]["partial0"]
        out[c // 4] += res.results[c]["partial1"]
    out += bo[None, None, :]
    return out


# revision 21
# speedup vs baseline: 1.0395x; 1.0395x over previous
"""Trainium2 Bass kernel for causal multi-head attention with adaptive
temperature (entropy-polynomial) softmax.

Problem shape: x [2, 2048, 1024], 16 heads x 64 dims, causal.
  q/k/v = x @ W{q,k,v}.T ; sim = q k^T / 8 (causal) ;
  attn = softmax(beta * sim), beta = f(entropy(softmax(sim))) ;
  out = (attn v) @ Wo.T + bo

Sharding (8 cores): core c owns batch b = c // 4 and heads
4*(c%4) .. 4*(c%4)+3.  Host sums the partials per batch and adds bo.

Device-side per core (m indexes head pairs {2m, 2m+1}):
  phase A   : qT16/kT16 = (W slice) @ x^T -> fp16 (qT pre-scaled 1/8);
              v packed fp16 + ones column per head (v_aug); x/W arrive
              fp16 from the host (halves input DMA)
  B1(m)     : entropy-stat sweep in [i, j] over [128,1024] psum tiles:
              scores -> exp (accum Z1) -> l*exp(l) (accum D)
  stats(m)  : H = ln Z1 - D/Z1 ; beta = where(H>.5, max(poly(H),1), 1)
  fold(m)   : PE transpose of beta + SBUF DMA gather + partition
              broadcast; qbT = qT16 * beta (fp16).  The transpose is
              emitted mid-way through the next PE-busy sweep so the
              cross-engine fold chain never stalls the PE.
  B2(m)     : transposed pass, group-outer: per group g (512 query
              cols) and head hh: scoresT[j,i] = kT16^T @ qbT (pairs of
              j blocks share one [128,1024] psum tile -> one exp),
              maskT on diagonal, exp -> e2 fp16, avT[65,512] +=
              v_aug^T @ e2 (row 64 = Z2 via ones col).  Per-group
              normalize (DMA-reshaped partition-parallel reciprocal)
              then the group's 4 rows of the output projection
              partial{m} = attT[m].T @ Wo^T stream out immediately.
Host sums partial0+partial1 over the 4 cores of each batch.
"""

import numpy as np

import concourse.bass as bass
import concourse.tile as tile
from concourse import bacc, mybir
from concourse.bass_utils import run_bass_kernel_spmd
from concourse.masks import make_identity

F32 = mybir.dt.float32
F32R = mybir.dt.float32r
BF16 = mybir.dt.bfloat16
FP16 = mybir.dt.float16
I32 = mybir.dt.int32
AFT = mybir.ActivationFunctionType
ALU = mybir.AluOpType

B, N, DIM = 2, 2048, 1024
H_TOT, HD = 16, 64
N_CORES = 8
NH = 4            # heads per core
CD = NH * HD      # 256 channel dims per core
NRB = N // 128    # 16 row blocks
NG = NRB // 4     # 4 groups of 4 row blocks (512 query cols each)
POLY = [-0.037, 0.481, -2.3, 4.917, -1.791]
MASK_VAL = -1e30
SCALE = 1.0 / 8.0  # 1/sqrt(64)

NUC = NRB * 2        # 32 (rb, head-in-pair) units per m


def build_kernel():
    nc = bacc.Bacc("TRN2", target_bir_lowering=False, debug=False,
                   num_devices=N_CORES)

    xT = nc.dram_tensor("xT", [DIM, N], FP16, kind="ExternalInput").ap()
    wqT = nc.dram_tensor("wqT", [DIM, CD], FP16, kind="ExternalInput").ap()
    wkT = nc.dram_tensor("wkT", [DIM, CD], FP16, kind="ExternalInput").ap()
    wvT = nc.dram_tensor("wvT", [DIM, CD], FP16, kind="ExternalInput").ap()
    woT = nc.dram_tensor("woT", [CD, DIM], F32, kind="ExternalInput").ap()
    maskin = nc.dram_tensor("maskin", [128, 128], F32, kind="ExternalInput").ap()
    maskTin = nc.dram_tensor("maskTin", [128, 128], F32, kind="ExternalInput").ap()
    partial = [nc.dram_tensor(f"partial{m}", [N, DIM], F32,
                              kind="ExternalOutput").ap() for m in range(2)]

    KC = DIM // 128  # 8 contraction chunks

    with tile.TileContext(nc) as tc:
        with tc.tile_pool(name="const", bufs=1) as constp, \
             tc.tile_pool(name="qkv_sb", bufs=1) as qkvp, \
             tc.tile_pool(name="attn_out", bufs=1) as aop, \
             tc.tile_pool(name="wo_sb", bufs=1) as wop, \
             tc.tile_pool(name="statsall", bufs=1) as sap:

            identf = constp.tile([128, 128], F32)
            make_identity(nc, identf[:])
            mask = constp.tile([128, 128], F32)
            nc.sync.dma_start(mask[:], maskin[:])
            maskT = constp.tile([128, 128], F32)
            nc.sync.dma_start(maskT[:], maskTin[:])
            ones32 = constp.tile([128, NUC], F32)
            nc.vector.memset(ones32[:], 1.0)

            qT16 = [qkvp.tile([128, N], FP16, tag=f"qT{m}", name=f"qT{m}") for m in range(2)]
            kT16 = [qkvp.tile([128, N], FP16, tag=f"kT{m}", name=f"kT{m}") for m in range(2)]
            vaug = [qkvp.tile([128, NH * 65], FP16, tag=f"va{j}", name=f"va{j}") for j in range(NRB)]
            qbT = [qkvp.tile([128, N], FP16, tag=f"qb{m}", name=f"qb{m}") for m in range(2)]
            attT = [aop.tile([128, N], F32R, tag=f"attT{m}", name=f"attT{m}") for m in range(2)]
            woS = [wop.tile([128, DIM], F32R, tag=f"wo{m}", name=f"wo{m}") for m in range(2)]

            # per-m stats accumulators: col = (rb*2 + hh)*2 + chunk
            Z1p = [sap.tile([128, 2 * NUC], F32, tag=f"Z1p{m}", name=f"Z1p{m}") for m in range(2)]
            D1p = [sap.tile([128, 2 * NUC], F32, tag=f"D1p{m}", name=f"D1p{m}") for m in range(2)]

            # ---- phase A: QKV projections ----
            with tc.tile_pool(name="xw_sb", bufs=1) as xwp, \
                 tc.tile_pool(name="qkv_ps", bufs=4, space="PSUM") as qkps:
                xTs = [xwp.tile([128, N], FP16, tag=f"xT{k}", name=f"xTs{k}") for k in range(KC)]
                wq_s = [xwp.tile([128, CD], FP16, tag=f"wq{k}", name=f"wq{k}") for k in range(KC)]
                wk_s = [xwp.tile([128, CD], FP16, tag=f"wk{k}", name=f"wk{k}") for k in range(KC)]
                wv_s = [xwp.tile([128, CD], FP16, tag=f"wv{k}", name=f"wv{k}") for k in range(KC)]
                for k in range(KC):
                    sl = slice(128 * k, 128 * (k + 1))
                    nc.sync.dma_start(wq_s[k][:], wqT[sl, :])
                    nc.sync.dma_start(wk_s[k][:], wkT[sl, :])
                    nc.sync.dma_start(wv_s[k][:], wvT[sl, :])
                    nc.sync.dma_start(xTs[k][:], xT[sl, :])
                for m in range(2):
                    nc.sync.dma_start(woS[m][:], woT[128 * m:128 * (m + 1), :].bitcast(F32R))

                for m in range(2):
                    for which, wt, dest, scl in (("q", wq_s, qT16, SCALE), ("k", wk_s, kT16, 1.0)):
                        for nn in range(N // 512):
                            pq = qkps.tile([128, 512], F32, tag="pq")
                            for k in range(KC):
                                nc.tensor.matmul(
                                    pq[:], wt[k][:, 128 * m:128 * (m + 1)],
                                    xTs[k][:, 512 * nn:512 * (nn + 1)],
                                    start=(k == 0), stop=(k == KC - 1))
                            if which == "q":
                                nc.scalar.activation(
                                    dest[m][:, 512 * nn:512 * (nn + 1)], pq[:],
                                    AFT.Copy, bias=0.0, scale=scl)
                            else:
                                nc.vector.tensor_copy(
                                    dest[m][:, 512 * nn:512 * (nn + 1)], pq[:])

                for jt in range(NRB):
                    pv = qkps.tile([128, CD], F32, tag="pv")
                    for k in range(KC):
                        nc.tensor.matmul(
                            pv[:], xTs[k][:, 128 * jt:128 * (jt + 1)], wv_s[k][:],
                            start=(k == 0), stop=(k == KC - 1))
                    va3 = vaug[jt][:].rearrange("p (h x) -> p h x", x=65)
                    nc.any.tensor_copy(
                        va3[:, :, 0:64],
                        pv[:].rearrange("p (h x) -> p h x", x=64))
                    nc.gpsimd.memset(va3[:, :, 64:65], 1.0)

            # ---- pipelined B1 / stats / fold / B2+C ----
            with tc.tile_pool(name="scr", bufs=4) as scrp, \
                 tc.tile_pool(name="stats", bufs=2) as stp, \
                 tc.tile_pool(name="bprep", bufs=2) as bpp, \
                 tc.tile_pool(name="e2sb", bufs=6) as e2p, \
                 tc.tile_pool(name="ostp", bufs=3) as ostp, \
                 tc.tile_pool(name="sc_ps", bufs=3, space="PSUM") as scps, \
                 tc.tile_pool(name="av_ps", bufs=2, space="PSUM") as avps:

                beta_sb = [None, None]

                def b1_sweep(m, inject=None):
                    nc.vector.memset(Z1p[m][:], 0.0)
                    nc.vector.memset(D1p[m][:], 0.0)
                    for rb in range(NRB):
                        if inject is not None and rb in inject:
                            inject[rb]()
                        W = 128 * (rb + 1)
                        tiles = [(o, min(1024, W - o)) for o in range(0, W, 1024)]
                        for hh in range(2):
                            base = 64 * hh
                            col = (rb * 2 + hh) * 2
                            q_l = qT16[m][base:base + 64, 128 * rb:128 * (rb + 1)]
                            for ci, (off, tw) in enumerate(tiles):
                                ps = scps.tile([128, 1024], F32, tag="ps_s")
                                for o2 in range(0, tw, 512):
                                    sw = min(512, tw - o2)
                                    nc.tensor.matmul(
                                        ps[:, o2:o2 + sw], q_l,
                                        kT16[m][base:base + 64,
                                                off + o2:off + o2 + sw],
                                        start=True, stop=True)
                                if off + tw == W:
                                    nc.vector.tensor_tensor(
                                        out=ps[:, tw - 128:tw],
                                        in0=ps[:, tw - 128:tw],
                                        in1=mask[:], op=ALU.add)
                                t1 = scrp.tile([128, 1024], F32, tag="t1")
                                nc.scalar.activation(
                                    t1[:, :tw], ps[:, :tw], AFT.Exp,
                                    bias=0.0, scale=1.0,
                                    accum_out=Z1p[m][:, col + ci:col + ci + 1])
                                s2 = scrp.tile([128, 1024], F32, tag="s2")
                                nc.vector.scalar_tensor_tensor(
                                    out=s2[:, :tw], in0=ps[:, :tw], scalar=1.0,
                                    in1=t1[:, :tw], op0=ALU.mult, op1=ALU.mult,
                                    accum_out=D1p[m][:, col + ci:col + ci + 1])

                def stats_chain(m):
                    Z1a = stp.tile([128, NUC], F32, tag="Z1a")
                    D1a = stp.tile([128, NUC], F32, tag="D1a")
                    nc.vector.tensor_reduce(
                        out=Z1a[:], in_=Z1p[m].rearrange("p (u c) -> p u c", c=2),
                        axis=mybir.AxisListType.X, op=ALU.add)
                    nc.vector.tensor_reduce(
                        out=D1a[:], in_=D1p[m].rearrange("p (u c) -> p u c", c=2),
                        axis=mybir.AxisListType.X, op=ALU.add)
                    rz = stp.tile([128, NUC], F32, tag="rz")
                    nc.vector.reciprocal(rz[:], Z1a[:])
                    dn = stp.tile([128, NUC], F32, tag="dn")
                    nc.vector.tensor_mul(dn[:], D1a[:], rz[:])
                    lnz = stp.tile([128, NUC], F32, tag="lnz")
                    nc.scalar.activation(lnz[:], Z1a[:], AFT.Ln, bias=0.0, scale=1.0)
                    Hent = stp.tile([128, NUC], F32, tag="Hent")
                    nc.vector.tensor_sub(Hent[:], lnz[:], dn[:])
                    p0 = stp.tile([128, NUC], F32, tag="p0")
                    nc.vector.tensor_scalar(out=p0[:], in0=Hent[:], scalar1=POLY[0],
                                            scalar2=POLY[1], op0=ALU.mult, op1=ALU.add)
                    p1 = stp.tile([128, NUC], F32, tag="p1")
                    for c in POLY[2:]:
                        nc.vector.tensor_mul(p1[:], p0[:], Hent[:])
                        nc.vector.tensor_scalar_add(p0[:], p1[:], c)
                    nc.vector.tensor_scalar_max(p1[:], p0[:], 1.0)
                    mk = stp.tile([128, NUC], I32, tag="mk")
                    nc.vector.tensor_scalar(out=mk[:], in0=Hent[:], scalar1=0.5,
                                            scalar2=None, op0=ALU.is_gt)
                    beta_m = stp.tile([128, NUC], F32, tag="beta_m")
                    nc.vector.tensor_copy(beta_m[:], ones32[:])
                    nc.vector.copy_predicated(beta_m[:], mk[:], p1[:])
                    beta_sb[m] = beta_m

                def fold(m):
                    beta_m = beta_sb[m]
                    btp = scps.tile([128, 1024], F32, tag="ps_s")
                    nc.tensor.transpose(btp[0:NUC, 0:128], beta_m[:], identf[:])
                    betaT = bpp.tile([NUC, 128], F32, tag="betaT")
                    nc.any.tensor_copy(betaT[:], btp[0:NUC, 0:128])
                    for hh in range(2):
                        base = 64 * hh
                        brow = bpp.tile([1, N], F32, tag="brow")
                        nc.sync.dma_start(brow[:], betaT[hh::2, :])
                        bb = bpp.tile([128, N], F32, tag="bb")
                        nc.gpsimd.partition_broadcast(bb[:], brow[:])
                        nc.vector.tensor_tensor(
                            out=qbT[m][base:base + 64, :],
                            in0=qT16[m][base:base + 64, :],
                            in1=bb[base:base + 64, :], op=ALU.mult)

                def proj_rows(m, g):
                    for rb in range(4 * g, 4 * g + 4):
                        pp = scps.tile([128, 1024], F32, tag="ps_s")
                        for nn in range(2):
                            nc.tensor.matmul(
                                pp[:, 512 * nn:512 * (nn + 1)],
                                attT[m][:, 128 * rb:128 * (rb + 1)],
                                woS[m][:, 512 * nn:512 * (nn + 1)],
                                start=True, stop=True)
                        ost = ostp.tile([128, 1024], F32, tag="ost")
                        nc.vector.tensor_copy(ost[:], pp[:])
                        nc.sync.dma_start(
                            partial[m][128 * rb:128 * (rb + 1), :], ost[:])

                def b2_sweep(m, inject=None):
                    for g in range(NG):
                        if inject is not None and g in inject:
                            inject[g]()
                        i0 = 512 * g
                        njt = 4 * g + 4
                        zall = stp.tile([1, 1024], F32, tag="zall")
                        for hh in range(2):
                            h = 2 * m + hh
                            base = 64 * hh
                            avp = avps.tile([65, 512], F32, tag="avp")
                            for b0 in range(0, njt, 4):
                                bjts = list(range(b0, min(b0 + 4, njt)))
                                e2s = {}
                                # pairs of j-blocks share one psum tile and
                                # one exp instruction
                                for pi in range(0, len(bjts), 2):
                                    pjts = bjts[pi:pi + 2]
                                    psT = scps.tile([128, 1024], F32, tag="ps_s")
                                    ew = e2p.tile([128, 1024], FP16, tag="e2")
                                    hi = 0
                                    for sub, jt in enumerate(pjts):
                                        off = max(0, 128 * (jt - 4 * g))
                                        w = 512 - off
                                        s0 = 512 * sub
                                        nc.tensor.matmul(
                                            psT[:, s0:s0 + w],
                                            kT16[m][base:base + 64,
                                                    128 * jt:128 * (jt + 1)],
                                            qbT[m][base:base + 64,
                                                   i0 + off:i0 + 512],
                                            start=True, stop=True)
                                        if jt >= 4 * g:
                                            nc.vector.tensor_tensor(
                                                out=psT[:, s0:s0 + 128],
                                                in0=psT[:, s0:s0 + 128],
                                                in1=maskT[:], op=ALU.add)
                                        e2s[jt] = (ew, s0, off, w)
                                        hi = s0 + w
                                    nc.scalar.activation(
                                        ew[:, 0:hi], psT[:, 0:hi], AFT.Exp,
                                        bias=0.0, scale=1.0)
                                for jt in bjts:
                                    ew, s0, off, w = e2s[jt]
                                    nc.tensor.matmul(
                                        avp[:, off:512],
                                        vaug[jt][:, 65 * h:65 * h + 65],
                                        ew[:, s0:s0 + w],
                                        start=(jt == 0), stop=(jt == njt - 1),
                                        skip_group_check=True)
                            nc.scalar.copy(
                                zall[0:1, 512 * hh:512 * (hh + 1)], avp[64:65, :])
                            avps_held[hh] = avp
                        # per-group normalize: reshape Z rows to [128, 8] via
                        # DMA for a partition-parallel reciprocal; multiply
                        # straight from the held psum into attT
                        zres = stp.tile([128, 8], F32, tag="zres")
                        nc.sync.dma_start(zres[:], zall[:])
                        zrr = stp.tile([128, 8], F32, tag="zrr")
                        nc.vector.reciprocal(zrr[:], zres[:])
                        zr = stp.tile([1, 1024], F32, tag="zr")
                        nc.sync.dma_start(zr[:], zrr[:])
                        for hh in range(2):
                            base = 64 * hh
                            rbv = stp.tile([128, 512], F32, tag="rbv")
                            nc.gpsimd.partition_broadcast(
                                rbv[:], zr[0:1, 512 * hh:512 * (hh + 1)])
                            nc.vector.tensor_tensor(
                                out=attT[m][base:base + 64, i0:i0 + 512],
                                in0=avps_held[hh][0:64, :],
                                in1=rbv[base:base + 64, :], op=ALU.mult)
                        proj_rows(m, g)

                avps_held = [None, None]

                b1_sweep(0)
                stats_chain(0)
                b1_sweep(1, inject={8: lambda: fold(0)})
                stats_chain(1)
                b2_sweep(0, inject={1: lambda: fold(1)})
                b2_sweep(1)

    nc.compile()
    return nc


_NC_CACHE = None
_LAST_IN_MAPS = None


def kernel(x, Wq, Wk, Wv, Wo, bo):
    global _NC_CACHE, _LAST_IN_MAPS
    x = np.asarray(x, dtype=np.float32)
    Wq = np.asarray(Wq, dtype=np.float32)
    Wk = np.asarray(Wk, dtype=np.float32)
    Wv = np.asarray(Wv, dtype=np.float32)
    Wo = np.asarray(Wo, dtype=np.float32)
    bo = np.asarray(bo, dtype=np.float32)

    if _NC_CACHE is None:
        _NC_CACHE = build_kernel()
    nc = _NC_CACHE

    mask_h = np.where(np.arange(128)[None, :] > np.arange(128)[:, None],
                      np.float32(MASK_VAL), np.float32(0.0)).astype(np.float32)
    maskT_h = np.ascontiguousarray(mask_h.T)
    woT_full = np.ascontiguousarray(Wo.T)  # [c, o]

    in_maps = []
    for c in range(N_CORES):
        b = c // 4
        s0 = CD * (c % 4)
        sl = slice(s0, s0 + CD)
        in_maps.append({
            "xT": np.ascontiguousarray(x[b].T.astype(np.float16)),
            "wqT": np.ascontiguousarray(Wq[sl, :].T.astype(np.float16)),
            "wkT": np.ascontiguousarray(Wk[sl, :].T.astype(np.float16)),
            "wvT": np.ascontiguousarray(Wv[sl, :].T.astype(np.float16)),
            "woT": np.ascontiguousarray(woT_full[sl, :]),
            "maskin": mask_h,
            "maskTin": maskT_h,
        })

    _LAST_IN_MAPS = in_maps
    res = run_bass_kernel_spmd(nc, in_maps, core_ids=list(range(N_CORES)))

    out = np.zeros((B, N, DIM), dtype=np.float32)
    for c in range(N_CORES):
        out[c // 4] += res.results[c]["partial0"]
        out[c // 4] += res.results[c]["partial1"]
    out += bo[None, None, :]
    return out


# revision 22
# speedup vs baseline: 1.1088x; 1.0667x over previous
"""Trainium2 Bass kernel for causal multi-head attention with adaptive
temperature (entropy-polynomial) softmax.

Problem shape: x [2, 2048, 1024], 16 heads x 64 dims, causal.
  q/k/v = x @ W{q,k,v}.T ; sim = q k^T / 8 (causal) ;
  attn = softmax(beta * sim), beta = f(entropy(softmax(sim))) ;
  out = (attn v) @ Wo.T + bo

Sharding (8 cores): core c owns batch b = c // 4 and heads
4*(c%4) .. 4*(c%4)+3.  Host sums the partials per batch and adds bo.

Device-side per core (m indexes head pairs {2m, 2m+1}):
  phase A   : qT16/kT16 = (W slice) @ x^T -> fp16 (qT pre-scaled 1/8);
              v packed fp16 + ones column per head (v_aug); x/W arrive
              fp16 from the host (halves input DMA)
  B1(m)     : entropy-stat sweep in [i, j] over [128,1024] psum tiles:
              scores -> exp (accum Z1) -> l*exp(l) (accum D)
  stats(m)  : H = ln Z1 - D/Z1 ; beta = where(H>.5, max(poly(H),1), 1)
  fold(m)   : PE transpose of beta + SBUF DMA gather + partition
              broadcast; qbT = qT16 * beta (fp16).  The transpose is
              emitted mid-way through the next PE-busy sweep so the
              cross-engine fold chain never stalls the PE.
  B2(m)     : transposed pass, group-outer: per group g (512 query
              cols) and head hh: scoresT[j,i] = kT16^T @ qbT (pairs of
              j blocks share one [128,1024] psum tile -> one exp),
              maskT on diagonal, exp -> e2 fp16, avT[65,512] +=
              v_aug^T @ e2 (row 64 = Z2 via ones col).  Per-group
              normalize (DMA-reshaped partition-parallel reciprocal)
              then the group's 4 rows of the output projection
              partial{m} = attT[m].T @ Wo^T stream out immediately.
Host sums partial0+partial1 over the 4 cores of each batch.
"""

import numpy as np

import concourse.bass as bass
import concourse.tile as tile
from concourse import bacc, mybir
from concourse.bass_utils import run_bass_kernel_spmd
from concourse.masks import make_identity

F32 = mybir.dt.float32
F32R = mybir.dt.float32r
BF16 = mybir.dt.bfloat16
FP16 = mybir.dt.float16
I32 = mybir.dt.int32
AFT = mybir.ActivationFunctionType
ALU = mybir.AluOpType

B, N, DIM = 2, 2048, 1024
H_TOT, HD = 16, 64
N_CORES = 8
NH = 4            # heads per core
CD = NH * HD      # 256 channel dims per core
NRB = N // 128    # 16 row blocks
NG = NRB // 4     # 4 groups of 4 row blocks (512 query cols each)
POLY = [-0.037, 0.481, -2.3, 4.917, -1.791]
MASK_VAL = -1e30
SCALE = 1.0 / 8.0  # 1/sqrt(64)

NUC = NRB * 2        # 32 (rb, head-in-pair) units per m


def build_kernel():
    nc = bacc.Bacc("TRN2", target_bir_lowering=False, debug=False,
                   num_devices=N_CORES)

    xT = nc.dram_tensor("xT", [DIM, N], FP16, kind="ExternalInput").ap()
    wqT = nc.dram_tensor("wqT", [DIM, CD], FP16, kind="ExternalInput").ap()
    wkT = nc.dram_tensor("wkT", [DIM, CD], FP16, kind="ExternalInput").ap()
    wvT = nc.dram_tensor("wvT", [DIM, CD], FP16, kind="ExternalInput").ap()
    woT = nc.dram_tensor("woT", [CD, DIM], F32, kind="ExternalInput").ap()
    maskin = nc.dram_tensor("maskin", [128, 128], F32, kind="ExternalInput").ap()
    maskTin = nc.dram_tensor("maskTin", [128, 128], F32, kind="ExternalInput").ap()
    partial = [nc.dram_tensor(f"partial{m}", [N, DIM], F32,
                              kind="ExternalOutput").ap() for m in range(2)]

    KC = DIM // 128  # 8 contraction chunks

    with tile.TileContext(nc) as tc:
        with tc.tile_pool(name="const", bufs=1) as constp, \
             tc.tile_pool(name="qkv_sb", bufs=1) as qkvp, \
             tc.tile_pool(name="attn_out", bufs=1) as aop, \
             tc.tile_pool(name="wo_sb", bufs=1) as wop, \
             tc.tile_pool(name="statsall", bufs=1) as sap:

            identf = constp.tile([128, 128], F32)
            make_identity(nc, identf[:])
            mask = constp.tile([128, 128], F32)
            nc.sync.dma_start(mask[:], maskin[:])
            maskT = constp.tile([128, 128], F32)
            nc.sync.dma_start(maskT[:], maskTin[:])
            ones32 = constp.tile([128, NUC], F32)
            nc.vector.memset(ones32[:], 1.0)

            qT16 = [qkvp.tile([128, N], FP16, tag=f"qT{m}", name=f"qT{m}") for m in range(2)]
            kT16 = [qkvp.tile([128, N], FP16, tag=f"kT{m}", name=f"kT{m}") for m in range(2)]
            vaug = [qkvp.tile([128, NH * 65], FP16, tag=f"va{j}", name=f"va{j}") for j in range(NRB)]
            qbT = [qkvp.tile([128, N], FP16, tag=f"qb{m}", name=f"qb{m}") for m in range(2)]
            attT = [aop.tile([128, N], F32R, tag=f"attT{m}", name=f"attT{m}") for m in range(2)]
            woS = [wop.tile([128, DIM], F32R, tag=f"wo{m}", name=f"wo{m}") for m in range(2)]

            # per-m stats accumulators: col = (rb*2 + hh)*2 + chunk
            Z1p = [sap.tile([128, 2 * NUC], F32, tag=f"Z1p{m}", name=f"Z1p{m}") for m in range(2)]
            D1p = [sap.tile([128, 2 * NUC], F32, tag=f"D1p{m}", name=f"D1p{m}") for m in range(2)]

            # ---- phase A: QKV projections ----
            with tc.tile_pool(name="xw_sb", bufs=1) as xwp, \
                 tc.tile_pool(name="qkv_ps", bufs=4, space="PSUM") as qkps:
                xTs = [xwp.tile([128, N], FP16, tag=f"xT{k}", name=f"xTs{k}") for k in range(KC)]
                wq_s = [xwp.tile([128, CD], FP16, tag=f"wq{k}", name=f"wq{k}") for k in range(KC)]
                wk_s = [xwp.tile([128, CD], FP16, tag=f"wk{k}", name=f"wk{k}") for k in range(KC)]
                wv_s = [xwp.tile([128, CD], FP16, tag=f"wv{k}", name=f"wv{k}") for k in range(KC)]
                for k in range(KC):
                    sl = slice(128 * k, 128 * (k + 1))
                    nc.sync.dma_start(wq_s[k][:], wqT[sl, :])
                    nc.sync.dma_start(wk_s[k][:], wkT[sl, :])
                    nc.sync.dma_start(wv_s[k][:], wvT[sl, :])
                    nc.sync.dma_start(xTs[k][:], xT[sl, :])
                for m in range(2):
                    nc.sync.dma_start(woS[m][:], woT[128 * m:128 * (m + 1), :].bitcast(F32R))

                for m in range(2):
                    for which, wt, dest, scl in (("q", wq_s, qT16, SCALE), ("k", wk_s, kT16, 1.0)):
                        for nn in range(N // 512):
                            pq = qkps.tile([128, 512], F32, tag="pq")
                            for k in range(KC):
                                nc.tensor.matmul(
                                    pq[:], wt[k][:, 128 * m:128 * (m + 1)],
                                    xTs[k][:, 512 * nn:512 * (nn + 1)],
                                    start=(k == 0), stop=(k == KC - 1))
                            if which == "q":
                                nc.scalar.activation(
                                    dest[m][:, 512 * nn:512 * (nn + 1)], pq[:],
                                    AFT.Copy, bias=0.0, scale=scl)
                            else:
                                nc.vector.tensor_copy(
                                    dest[m][:, 512 * nn:512 * (nn + 1)], pq[:])

                for jt in range(NRB):
                    pv = qkps.tile([128, CD], F32, tag="pv")
                    for k in range(KC):
                        nc.tensor.matmul(
                            pv[:], xTs[k][:, 128 * jt:128 * (jt + 1)], wv_s[k][:],
                            start=(k == 0), stop=(k == KC - 1))
                    va3 = vaug[jt][:].rearrange("p (h x) -> p h x", x=65)
                    nc.any.tensor_copy(
                        va3[:, :, 0:64],
                        pv[:].rearrange("p (h x) -> p h x", x=64))
                    nc.gpsimd.memset(va3[:, :, 64:65], 1.0)

            # ---- pipelined B1 / stats / fold / B2+C ----
            with tc.tile_pool(name="scr", bufs=4) as scrp, \
                 tc.tile_pool(name="stats", bufs=2) as stp, \
                 tc.tile_pool(name="bprep", bufs=2) as bpp, \
                 tc.tile_pool(name="e2sb", bufs=6) as e2p, \
                 tc.tile_pool(name="ostp", bufs=3) as ostp, \
                 tc.tile_pool(name="sc_ps", bufs=3, space="PSUM") as scps, \
                 tc.tile_pool(name="av_ps", bufs=2, space="PSUM") as avps:

                beta_sb = [None, None]

                def b1_sweep(m, inject=None):
                    nc.vector.memset(Z1p[m][:], 0.0)
                    nc.vector.memset(D1p[m][:], 0.0)
                    for rb in range(NRB):
                        if inject is not None and rb in inject:
                            inject[rb]()
                        W = 128 * (rb + 1)
                        tiles = [(o, min(1024, W - o)) for o in range(0, W, 1024)]
                        for hh in range(2):
                            base = 64 * hh
                            col = (rb * 2 + hh) * 2
                            q_l = qT16[m][base:base + 64, 128 * rb:128 * (rb + 1)]
                            for ci, (off, tw) in enumerate(tiles):
                                ps = scps.tile([128, 1024], F32, tag="ps_s")
                                for o2 in range(0, tw, 512):
                                    sw = min(512, tw - o2)
                                    nc.tensor.matmul(
                                        ps[:, o2:o2 + sw], q_l,
                                        kT16[m][base:base + 64,
                                                off + o2:off + o2 + sw],
                                        start=True, stop=True)
                                if off + tw == W:
                                    nc.vector.tensor_tensor(
                                        out=ps[:, tw - 128:tw],
                                        in0=ps[:, tw - 128:tw],
                                        in1=mask[:], op=ALU.add)
                                t1 = scrp.tile([128, 1024], F32, tag="t1")
                                nc.scalar.activation(
                                    t1[:, :tw], ps[:, :tw], AFT.Exp,
                                    bias=0.0, scale=1.0,
                                    accum_out=Z1p[m][:, col + ci:col + ci + 1])
                                s2 = scrp.tile([128, 1024], F32, tag="s2")
                                nc.vector.scalar_tensor_tensor(
                                    out=s2[:, :tw], in0=ps[:, :tw], scalar=1.0,
                                    in1=t1[:, :tw], op0=ALU.mult, op1=ALU.mult,
                                    accum_out=D1p[m][:, col + ci:col + ci + 1])

                def stats_chain(m):
                    Z1a = stp.tile([128, NUC], F32, tag="Z1a")
                    D1a = stp.tile([128, NUC], F32, tag="D1a")
                    nc.vector.tensor_reduce(
                        out=Z1a[:], in_=Z1p[m].rearrange("p (u c) -> p u c", c=2),
                        axis=mybir.AxisListType.X, op=ALU.add)
                    nc.vector.tensor_reduce(
                        out=D1a[:], in_=D1p[m].rearrange("p (u c) -> p u c", c=2),
                        axis=mybir.AxisListType.X, op=ALU.add)
                    rz = stp.tile([128, NUC], F32, tag="rz")
                    nc.vector.reciprocal(rz[:], Z1a[:])
                    dn = stp.tile([128, NUC], F32, tag="dn")
                    nc.vector.tensor_mul(dn[:], D1a[:], rz[:])
                    lnz = stp.tile([128, NUC], F32, tag="lnz")
                    nc.scalar.activation(lnz[:], Z1a[:], AFT.Ln, bias=0.0, scale=1.0)
                    Hent = stp.tile([128, NUC], F32, tag="Hent")
                    nc.vector.tensor_sub(Hent[:], lnz[:], dn[:])
                    p0 = stp.tile([128, NUC], F32, tag="p0")
                    nc.vector.tensor_scalar(out=p0[:], in0=Hent[:], scalar1=POLY[0],
                                            scalar2=POLY[1], op0=ALU.mult, op1=ALU.add)
                    p1 = stp.tile([128, NUC], F32, tag="p1")
                    for c in POLY[2:]:
                        nc.vector.tensor_mul(p1[:], p0[:], Hent[:])
                        nc.vector.tensor_scalar_add(p0[:], p1[:], c)
                    nc.vector.tensor_scalar_max(p1[:], p0[:], 1.0)
                    mk = stp.tile([128, NUC], I32, tag="mk")
                    nc.vector.tensor_scalar(out=mk[:], in0=Hent[:], scalar1=0.5,
                                            scalar2=None, op0=ALU.is_gt)
                    beta_m = stp.tile([128, NUC], F32, tag="beta_m")
                    nc.vector.tensor_copy(beta_m[:], ones32[:])
                    nc.vector.copy_predicated(beta_m[:], mk[:], p1[:])
                    beta_sb[m] = beta_m

                def fold(m):
                    beta_m = beta_sb[m]
                    btp = scps.tile([128, 1024], F32, tag="ps_s")
                    nc.tensor.transpose(btp[0:NUC, 0:128], beta_m[:], identf[:])
                    betaT = bpp.tile([NUC, 128], F32, tag="betaT")
                    nc.any.tensor_copy(betaT[:], btp[0:NUC, 0:128])
                    for hh in range(2):
                        base = 64 * hh
                        brow = bpp.tile([1, N], F32, tag="brow")
                        nc.sync.dma_start(brow[:], betaT[hh::2, :])
                        bb = bpp.tile([128, N], F32, tag="bb")
                        nc.gpsimd.partition_broadcast(bb[:], brow[:])
                        nc.vector.tensor_tensor(
                            out=qbT[m][base:base + 64, :],
                            in0=qT16[m][base:base + 64, :],
                            in1=bb[base:base + 64, :], op=ALU.mult)

                def proj_rows(m, g):
                    for rb in range(4 * g, 4 * g + 4):
                        pp = scps.tile([128, 1024], F32, tag="ps_s")
                        for nn in range(2):
                            nc.tensor.matmul(
                                pp[:, 512 * nn:512 * (nn + 1)],
                                attT[m][:, 128 * rb:128 * (rb + 1)],
                                woS[m][:, 512 * nn:512 * (nn + 1)],
                                start=True, stop=True)
                        ost = ostp.tile([128, 1024], F32, tag="ost")
                        nc.vector.tensor_copy(ost[:], pp[:])
                        nc.sync.dma_start(
                            partial[m][128 * rb:128 * (rb + 1), :], ost[:])

                def b2_sweep(m, inject=None):
                    for g in range(NG):
                        if inject is not None and g in inject:
                            inject[g]()
                        i0 = 512 * g
                        njt = 4 * g + 4
                        zall = stp.tile([1, 1024], F32, tag="zall")
                        for hh in range(2):
                            h = 2 * m + hh
                            base = 64 * hh
                            avp = avps.tile([65, 512], F32, tag="avp")
                            for b0 in range(0, njt, 4):
                                bjts = list(range(b0, min(b0 + 4, njt)))
                                e2s = {}
                                # pairs of j-blocks share one psum tile and
                                # one exp instruction
                                for pi in range(0, len(bjts), 2):
                                    pjts = bjts[pi:pi + 2]
                                    psT = scps.tile([128, 1024], F32, tag="ps_s")
                                    ew = e2p.tile([128, 1024], FP16, tag="e2")
                                    hi = 0
                                    for sub, jt in enumerate(pjts):
                                        off = max(0, 128 * (jt - 4 * g))
                                        w = 512 - off
                                        s0 = 512 * sub
                                        nc.tensor.matmul(
                                            psT[:, s0:s0 + w],
                                            kT16[m][base:base + 64,
                                                    128 * jt:128 * (jt + 1)],
                                            qbT[m][base:base + 64,
                                                   i0 + off:i0 + 512],
                                            start=True, stop=True)
                                        if jt >= 4 * g:
                                            nc.vector.tensor_tensor(
                                                out=psT[:, s0:s0 + 128],
                                                in0=psT[:, s0:s0 + 128],
                                                in1=maskT[:], op=ALU.add)
                                        e2s[jt] = (ew, s0, off, w)
                                        hi = s0 + w
                                    nc.scalar.activation(
                                        ew[:, 0:hi], psT[:, 0:hi], AFT.Exp,
                                        bias=0.0, scale=1.0)
                                for jt in bjts:
                                    ew, s0, off, w = e2s[jt]
                                    nc.tensor.matmul(
                                        avp[:, off:512],
                                        vaug[jt][:, 65 * h:65 * h + 65],
                                        ew[:, s0:s0 + w],
                                        start=(jt == 0), stop=(jt == njt - 1),
                                        skip_group_check=True)
                            nc.vector.tensor_copy(
                                zall[0:1, 512 * hh:512 * (hh + 1)], avp[64:65, :])
                            nc.vector.tensor_copy(
                                attT[m][base:base + 64, i0:i0 + 512], avp[0:64, :])
                        # per-group normalize: reshape Z rows to [128, 8] via
                        # DMA for a partition-parallel reciprocal; multiply
                        # straight from the held psum into attT
                        zres = stp.tile([128, 8], F32, tag="zres")
                        nc.sync.dma_start(zres[:], zall[:])
                        zrr = stp.tile([128, 8], F32, tag="zrr")
                        nc.vector.reciprocal(zrr[:], zres[:])
                        zr = stp.tile([1, 1024], F32, tag="zr")
                        nc.sync.dma_start(zr[:], zrr[:])
                        for hh in range(2):
                            base = 64 * hh
                            rbv = stp.tile([128, 512], F32, tag="rbv")
                            nc.gpsimd.partition_broadcast(
                                rbv[:], zr[0:1, 512 * hh:512 * (hh + 1)])
                            att_sl = attT[m][base:base + 64, i0:i0 + 512]
                            nc.vector.tensor_tensor(
                                out=att_sl, in0=att_sl.bitcast(F32),
                                in1=rbv[base:base + 64, :], op=ALU.mult)
                        proj_rows(m, g)

                b1_sweep(0)
                stats_chain(0)
                b1_sweep(1, inject={8: lambda: fold(0)})
                stats_chain(1)
                b2_sweep(0, inject={1: lambda: fold(1)})
                b2_sweep(1)

    nc.compile()
    return nc


_NC_CACHE = None
_LAST_IN_MAPS = None


def kernel(x, Wq, Wk, Wv, Wo, bo):
    global _NC_CACHE, _LAST_IN_MAPS
    x = np.asarray(x, dtype=np.float32)
    Wq = np.asarray(Wq, dtype=np.float32)
    Wk = np.asarray(Wk, dtype=np.float32)
    Wv = np.asarray(Wv, dtype=np.float32)
    Wo = np.asarray(Wo, dtype=np.float32)
    bo = np.asarray(bo, dtype=np.float32)

    if _NC_CACHE is None:
        _NC_CACHE = build_kernel()
    nc = _NC_CACHE

    mask_h = np.where(np.arange(128)[None, :] > np.arange(128)[:, None],
                      np.float32(MASK_VAL), np.float32(0.0)).astype(np.float32)
    maskT_h = np.ascontiguousarray(mask_h.T)
    woT_full = np.ascontiguousarray(Wo.T)  # [c, o]

    in_maps = []
    for c in range(N_CORES):
        b = c // 4
        s0 = CD * (c % 4)
        sl = slice(s0, s0 + CD)
        in_maps.append({
            "xT": np.ascontiguousarray(x[b].T.astype(np.float16)),
            "wqT": np.ascontiguousarray(Wq[sl, :].T.astype(np.float16)),
            "wkT": np.ascontiguousarray(Wk[sl, :].T.astype(np.float16)),
            "wvT": np.ascontiguousarray(Wv[sl, :].T.astype(np.float16)),
            "woT": np.ascontiguousarray(woT_full[sl, :]),
            "maskin": mask_h,
            "maskTin": maskT_h,
        })

    _LAST_IN_MAPS = in_maps
    res = run_bass_kernel_spmd(nc, in_maps, core_ids=list(range(N_CORES)))

    out = np.zeros((B, N, DIM), dtype=np.float32)
    for c in range(N_CORES):
        out[c // 4] += res.results[c]["partial0"]
        out[c // 4] += res.results[c]["partial1"]
    out += bo[None, None, :]
    return out
